# revision 21
# baseline (speedup 1.0000x reference)
"""Trainium2 Bass kernel for a 2-layer GCN fingerprint network.

    h   = relu(x @ W_i + b_i)                  [N, 128] -> [N, 64]
    z   = gcn_conv(h, edge_index, W_c)         scatter/gather over E edges
    h2  = relu(z @ W_h + b_h)
    out = h2 @ W_o + b_o                       [N, 1]

Strategy v2.1 (8 NeuronCores, full input in / full output out):

The graph is known at kernel() time, so ALL data-dependent routing is done
on the host: the host pre-orders x columns into "slot-sequence" order and
the device recomputes h per EDGE (no gather descriptors at all; the v1
dma_gather design was bottlenecked by Pool-engine descriptor generation).

  - per-edge norm factors into per-node scales: with dis = deg^-0.5,
      z_d = dis_d * sum_{e: col(e)=d} dis_src * relu(x[src] @ W_i + b_i) @ W_c
  - dis_src > 0 folds through the relu (relu(c*u) = c*relu(u)): the host
    bakes dis_src into x.  Nonzero b_i is handled by a rank-1
    (contraction-1) matmul accumulating b_i (x) disRow into the PSUM.
  - no nonlinearity sits between W_c and W_h, so W_ch = W_c @ W_h is
    precomputed on the host; the per-dst dis_d scale commutes to the very
    end (relu(c*v + b) = c*relu(v + b/c)).
  - destinations are sorted by in-degree and grouped into 128-dst blocks;
    block j gets K_j slots (max in-degree over the 8 blocks dealt at step
    j; schedule shared by all cores so the SPMD program is identical).
  - A/B partition packing: slots are split into an A half and a B half.
    One PSUM tile [128, 512] holds h for 512 A-entries on partitions 0:64
    and 512 B-entries on partitions 64:128, via two accumulating matmuls
    with zero-padded stationaries [W_i | 0] and [0 | W_i].  All downstream
    vector/scalar ops then run at full 128-partition width.
  - relu + segment-sum are fused: scalar_tensor_tensor computes
    AG += max(psum, 0) per chunk (bf16 accumulator), then two halving adds
    collapse AG's 4 slot-columns; the A-half/B-half merge is folded into
    the tail matmul with a stacked stationary [W_ch ; W_ch].
  - the tail (W_ch, relu, W_o, * dis_d) runs once, batched over all 49
    blocks at 512-wide, entirely in bf16 (fp32 matmuls are 4x slower).

Per-core traffic is the ~28MB xseq stream; everything else is on-chip.
"""

import sys

sys.path.insert(0, "/opt/trn_rl_repo")

from contextlib import ExitStack

import ml_dtypes
import numpy as np

import concourse.bass as bass
import concourse.tile as tile
from concourse import bacc, mybir
from concourse.bass_utils import run_bass_kernel_spmd

F32 = mybir.dt.float32
BF16 = mybir.dt.bfloat16
AF = mybir.ActivationFunctionType
ALU = mybir.AluOpType

N_CORES = 8
P = 128
MMF = 512          # matmul moving free dim (4 slots of 128)


def _host_prep(x, edge_index, W_i, b_i, W_c, W_h, b_h, W_o, b_o):
    """Returns (in_maps, meta) for run_bass_kernel_spmd."""
    n, in_dim = x.shape
    hid = W_i.shape[1]
    npad = -(-n // 1024) * 1024
    nblkg = npad // P
    assert nblkg % N_CORES == 0
    nblk = nblkg // N_CORES

    row = np.concatenate([edge_index[0], np.arange(n)]).astype(np.int64)
    col = np.concatenate([edge_index[1], np.arange(n)]).astype(np.int64)

    outdeg = np.bincount(row, minlength=n).astype(np.float64)
    dis = (outdeg ** -0.5).astype(np.float32)   # deg >= 1 (self loops)

    indeg = np.bincount(col, minlength=npad)
    order = np.argsort(-indeg, kind="stable")   # dsts by in-degree desc
    dst_gp = order.reshape(nblkg, P)            # [global block, partition]
    kblk = indeg[order].reshape(nblkg, P).max(1)
    # blocks are in degree order; deal round-robin: step j gets blocks
    # j*8 .. j*8+7, K_j = max over them (tight since sorted)
    K = kblk.reshape(nblk, N_CORES).max(1).astype(np.int64)
    K = np.maximum(K, 1)
    # processing order: K descending (deep prefetch while compute ramps,
    # small blocks drain quickly at the end); a pyramid order measured worse
    perm = np.arange(nblk)
    K = K[perm]
    KH = -(-K // 2)                              # A/B pair-slots per block
    CW = 2 * KH * P                              # xseq columns per block
    cbase = np.concatenate([[0], np.cumsum(CW)])
    Ltot = int(cbase[-1])

    # edges sorted by destination; starts[d] = first edge of dst d
    e_order = np.argsort(col, kind="stable")
    csrc = row[e_order]
    starts = np.searchsorted(col[e_order], np.arange(npad))

    # per-(step, slot) tables, slot s of block j lives at xseq column
    #   cbase[j] + (sp // 4) * 1024 + half * wt + (sp % 4) * 128 + p
    # where sp = s if s < KH[j] (A half) else s - KH[j] (B half) and wt is
    # the chunk width (512, except the last partial chunk of a block)
    SKtot = int(K.sum())
    row_j = np.repeat(np.arange(nblk), K)            # [SKtot]
    row_s = np.arange(SKtot) - np.repeat(np.cumsum(K) - K, K)
    khj = KH[row_j]
    half = (row_s >= khj).astype(np.int64)
    sp = row_s - half * khj                          # pair-slot index
    wt = np.minimum(MMF, (khj - (sp // 4) * 4) * P)  # chunk width
    colpos = cbase[row_j] + (sp // 4) * 1024 + half * wt + (sp % 4) * P

    # dis-prescaled, transposed x with a zero pad column at index n
    xs_T = np.zeros((in_dim, n + 1), ml_dtypes.bfloat16)
    xs_T[:, :n] = (x.T * dis[None, :]).astype(ml_dtypes.bfloat16)

    dis_pad = np.zeros(npad, np.float32)
    dis_pad[:n] = dis

    has_bi = bool(np.any(np.asarray(b_i)))
    has_bh = bool(np.any(np.asarray(b_h)))

    in_maps = []
    gbs = []
    for c in range(N_CORES):
        gb = perm * N_CORES + c                      # global block ids
        gbs.append(gb)
        dsts = dst_gp[gb]                            # [nblk, P]
        dst_mat = dsts[row_j]                        # [SKtot, P]
        deg_mat = indeg[dst_mat]
        mask = row_s[:, None] < deg_mat              # valid slot?
        eidx = starts[dst_mat] + row_s[:, None]
        seq = np.where(mask, csrc[np.minimum(eidx, len(csrc) - 1)], n)
        seq_cols = np.full(Ltot, n, np.int64)        # default: zero pad col
        seq_cols[(colpos[:, None] + np.arange(P)).reshape(-1)] = seq.reshape(-1)
        xseq = np.ascontiguousarray(xs_T[:, seq_cols])
        dRow = dis_pad[dsts].reshape(1, nblk * P).astype(np.float32)
        m = {"xseq": xseq, "dRow": np.ascontiguousarray(dRow)}
        if has_bi:
            dseq_e = np.where(mask, dis[np.minimum(seq, n - 1)], 0.0)
            dseq = np.zeros(Ltot, np.float32)
            dseq[(colpos[:, None] + np.arange(P)).reshape(-1)] = dseq_e.reshape(-1)
            m["disSeq"] = dseq.reshape(1, Ltot)
        if has_bh:
            with np.errstate(divide="ignore"):
                invd = np.where(dRow > 0, 1.0 / np.maximum(dRow, 1e-30), 0.0)
            m["invdRow"] = invd.astype(np.float32)
        in_maps.append(m)

    W_ch = (np.asarray(W_c, np.float64) @ np.asarray(W_h, np.float64))
    Wi64 = np.asarray(W_i, np.float64)
    W_iA = np.concatenate([Wi64, np.zeros_like(Wi64)], axis=1)  # [W_i | 0]
    W_iB = np.concatenate([np.zeros_like(Wi64), Wi64], axis=1)  # [0 | W_i]
    W_chAB = np.concatenate([W_ch, W_ch], axis=0)               # [W_ch ; W_ch]
    shared = {
        "W_iA": np.ascontiguousarray(W_iA).astype(ml_dtypes.bfloat16),
        "W_iB": np.ascontiguousarray(W_iB).astype(ml_dtypes.bfloat16),
        "W_chAB": np.ascontiguousarray(W_chAB).astype(ml_dtypes.bfloat16),
        "W_o": np.asarray(W_o).astype(ml_dtypes.bfloat16),
    }
    if has_bi:
        shared["b_i"] = np.asarray(b_i, np.float32).reshape(1, hid)
    if has_bh:
        shared["b_h"] = np.asarray(b_h, np.float32).reshape(1, hid)
    for m in in_maps:
        m.update(shared)

    meta = {
        "n": n,
        "npad": npad,
        "nblk": nblk,
        "K": K,
        "KH": KH,
        "cbase": cbase,
        "Ltot": Ltot,
        "in_dim": in_dim,
        "hid": hid,
        "dst_gp": dst_gp,
        "gbs": gbs,
        "has_bi": has_bi,
        "has_bh": has_bh,
        "b_o": float(np.asarray(b_o).reshape(-1)[0]),
    }
    return in_maps, meta


def _build(meta):
    nblk = meta["nblk"]
    KH = meta["KH"]
    cbase = meta["cbase"]
    Ltot = meta["Ltot"]
    in_dim = meta["in_dim"]
    hid = meta["hid"]
    has_bi = meta["has_bi"]
    has_bh = meta["has_bh"]
    b_o = meta["b_o"]
    khmax = int(KH.max())
    NO = nblk * P                                  # output columns

    nc = bacc.Bacc()
    xseq = nc.declare_dram_parameter("xseq", [in_dim, Ltot], BF16, isOutput=False)
    W_iA = nc.declare_dram_parameter("W_iA", [in_dim, 2 * hid], BF16, isOutput=False)
    W_iB = nc.declare_dram_parameter("W_iB", [in_dim, 2 * hid], BF16, isOutput=False)
    W_chAB = nc.declare_dram_parameter("W_chAB", [2 * hid, hid], BF16,
                                       isOutput=False)
    W_o = nc.declare_dram_parameter("W_o", [hid, 1], BF16, isOutput=False)
    dRow = nc.declare_dram_parameter("dRow", [1, NO], F32, isOutput=False)
    if has_bi:
        b_i = nc.declare_dram_parameter("b_i", [1, hid], F32, isOutput=False)
        disSeq = nc.declare_dram_parameter("disSeq", [1, Ltot], F32, isOutput=False)
    if has_bh:
        b_h = nc.declare_dram_parameter("b_h", [1, hid], F32, isOutput=False)
        invdRow = nc.declare_dram_parameter("invdRow", [1, NO], F32, isOutput=False)
    out = nc.declare_dram_parameter("out", [1, NO], F32, isOutput=True)

    with tile.TileContext(nc) as tc, ExitStack() as ctx:
        singles = ctx.enter_context(tc.tile_pool(name="singles", bufs=1))
        sWiA = singles.tile([in_dim, 2 * hid], BF16)
        sWiB = singles.tile([in_dim, 2 * hid], BF16)
        sWch = singles.tile([2 * hid, hid], BF16)
        sWo = singles.tile([hid, 1], BF16)
        sdR = singles.tile([1, NO], F32)
        zall = singles.tile([2 * hid, NO], BF16)
        outrow = singles.tile([1, NO], F32)
        loads = [(sWiA, W_iA), (sWiB, W_iB), (sWch, W_chAB), (sWo, W_o),
                 (sdR, dRow)]
        if has_bi:
            sbi = singles.tile([1, hid], F32)
            sdis = singles.tile([1, Ltot], F32)
            loads += [(sbi, b_i), (sdis, disSeq)]
        if has_bh:
            sbh = singles.tile([1, hid], F32)
            sinvd = singles.tile([1, NO], F32)
            loads += [(sbh, b_h), (sinvd, invdRow)]
        for dst_t, src_t in loads:
            nc.sync.dma_start(out=dst_t[:], in_=src_t[:])

        with (
            tc.tile_pool(name="px", bufs=5) as px,
            tc.tile_pool(name="pps", bufs=4, space="PSUM") as pps,
            tc.tile_pool(name="pag", bufs=3) as pag,
            tc.tile_pool(name="ph", bufs=2) as ph,
            tc.tile_pool(name="ps2", bufs=2, space="PSUM") as ps2,
            tc.tile_pool(name="pso", bufs=2, space="PSUM") as pso,
        ):
            def tail_chunk(t):
                # tail over 4 blocks: W_ch (+A/B merge), relu, W_o, *dis
                w = min(MMF, NO - t)
                p2 = ps2.tile([hid, MMF], F32)
                nc.tensor.matmul(p2[:, :w], lhsT=sWch[:], rhs=zall[:, t: t + w],
                                 start=True, stop=not has_bh)
                if has_bh:
                    nc.tensor.matmul(p2[:, :w], lhsT=sbh[:],
                                     rhs=sinvd[:, t: t + w],
                                     start=False, stop=True)
                h2 = ph.tile([hid, MMF], BF16)
                nc.scalar.activation(h2[:, :w], p2[:, :w], AF.Relu, bias=0.0)
                po = pso.tile([1, MMF], F32)
                nc.tensor.matmul(po[:, :w], lhsT=sWo[:], rhs=h2[:, :w],
                                 start=True, stop=True)
                nc.vector.tensor_mul(outrow[:, t: t + w], po[:, :w],
                                     sdR[:, t: t + w])
                if b_o != 0.0:
                    nc.vector.tensor_scalar_add(
                        outrow[:, t: t + w], outrow[:, t: t + w], b_o,
                    )

            for j in range(nblk):
                KHj = int(KH[j])
                off = int(cbase[j])
                L = 2 * KHj * P                    # block columns (A+B)
                xb = px.tile([in_dim, 2 * khmax * P], BF16, tag="xb")
                qs = (nc.sync, nc.scalar, nc.gpsimd)
                nch_j = -(-KHj // 4)
                cmid = (nch_j // 2) * 1024         # split at a chunk boundary
                if cmid > 0:
                    qs[j % 3].dma_start(out=xb[:, :cmid],
                                        in_=xseq[:, off: off + cmid])
                    qs[(j + 1) % 3].dma_start(out=xb[:, cmid:L],
                                              in_=xseq[:, off + cmid: off + L])
                else:
                    qs[j % 3].dma_start(out=xb[:, :L], in_=xseq[:, off: off + L])
                # Pool cannot read PSUM, so the STT chunks stay on DVE; the
                # SBUF-only collapse ops go to the otherwise-idle Pool engine
                ve = nc.gpsimd
                AG = pag.tile([P, MMF], BF16, tag="ag")
                nchunk = -(-KHj // 4)
                for t in range(nchunk):
                    w = min(MMF, KHj * P - t * MMF)
                    ca = t * 1024                  # A cols of this chunk
                    ps = pps.tile([P, MMF], F32)
                    nc.tensor.matmul(
                        ps[:, :w], lhsT=sWiA[:], rhs=xb[:, ca: ca + w],
                        start=True, stop=False,
                    )
                    nc.tensor.matmul(
                        ps[:, :w], lhsT=sWiB[:],
                        rhs=xb[:, ca + w: ca + 2 * w],
                        start=False, stop=not has_bi,
                    )
                    if has_bi:
                        # rank-1 bias: A then B half (disSeq is column-matched)
                        nc.tensor.matmul(
                            ps[:, :w], lhsT=sbi[:],
                            rhs=sdis[:, off + ca: off + ca + w],
                            start=False, stop=False,
                        )
                        nc.tensor.matmul(
                            ps[:, :w], lhsT=sbi[:],
                            rhs=sdis[:, off + ca + w: off + ca + 2 * w],
                            start=False, stop=True,
                        )
                    if t == 0:
                        nc.scalar.activation(AG[:, :w], ps[:, :w],
                                             AF.Relu, bias=0.0)
                    else:
                        nc.vector.scalar_tensor_tensor(
                            AG[:, :w], ps[:, :w], 0.0, AG[:, :w],
                            op0=ALU.max, op1=ALU.add,
                        )
                # collapse AG's remaining (up to 4) slot-columns; the last
                # halving step (on Pool, which is otherwise just issuing
                # DMAs) writes straight into zall
                zsl = zall[:, j * P: (j + 1) * P]
                k = min(KHj, 4)
                while k > 2:
                    k2 = k // 2
                    h = k - k2
                    nc.vector.tensor_add(
                        AG[:, : k2 * P], AG[:, : k2 * P],
                        AG[:, h * P: (h + k2) * P],
                    )
                    k = h
                if k == 2:
                    ve.tensor_add(zsl, AG[:, :P], AG[:, P: 2 * P])
                else:
                    ve.tensor_copy(zsl, AG[:, :P])
                if j % 4 == 3:
                    tail_chunk((j - 3) * P)
            if nblk % 4 != 0:
                tail_chunk((nblk - nblk % 4) * P)
        nc.sync.dma_start(out=out[:], in_=outrow[:])

    nc.finalize()
    return nc


def _assemble(results, meta):
    n = meta["n"]
    npad = meta["npad"]
    nblk = meta["nblk"]
    dst_gp = meta["dst_gp"]
    out_full = np.zeros(npad, np.float32)
    for c in range(N_CORES):
        vals = np.asarray(results[c]["out"]).reshape(nblk * P)
        out_full[dst_gp[meta["gbs"][c]].ravel()] = vals
    return out_full[:n].reshape(n, 1).astype(np.float32)


def kernel(x, edge_index, W_i, b_i, W_c, W_h, b_h, W_o, b_o):
    x = np.asarray(x)
    edge_index = np.asarray(edge_index)
    in_maps, meta = _host_prep(
        x, edge_index,
        np.asarray(W_i), np.asarray(b_i), np.asarray(W_c),
        np.asarray(W_h), np.asarray(b_h), np.asarray(W_o), np.asarray(b_o),
    )
    nc = _build(meta)
    res = run_bass_kernel_spmd(nc, in_maps, list(range(N_CORES)))
    return _assemble(res.results, meta)


# revision 23
# speedup vs baseline: 1.1666x; 1.1666x over previous
"""Trainium2 Bass kernel for a 2-layer GCN fingerprint network.

    h   = relu(x @ W_i + b_i)                  [N, 128] -> [N, 64]
    z   = gcn_conv(h, edge_index, W_c)         scatter/gather over E edges
    h2  = relu(z @ W_h + b_h)
    out = h2 @ W_o + b_o                       [N, 1]

Strategy v2.1 (8 NeuronCores, full input in / full output out):

The graph is known at kernel() time, so ALL data-dependent routing is done
on the host: the host pre-orders x columns into "slot-sequence" order and
the device recomputes h per EDGE (no gather descriptors at all; the v1
dma_gather design was bottlenecked by Pool-engine descriptor generation).

  - per-edge norm factors into per-node scales: with dis = deg^-0.5,
      z_d = dis_d * sum_{e: col(e)=d} dis_src * relu(x[src] @ W_i + b_i) @ W_c
  - dis_src > 0 folds through the relu (relu(c*u) = c*relu(u)): the host
    bakes dis_src into x.  Nonzero b_i is handled by a rank-1
    (contraction-1) matmul accumulating b_i (x) disRow into the PSUM.
  - no nonlinearity sits between W_c and W_h, so W_ch = W_c @ W_h is
    precomputed on the host; the per-dst dis_d scale commutes to the very
    end (relu(c*v + b) = c*relu(v + b/c)).
  - destinations are sorted by in-degree and grouped into 128-dst blocks;
    block j gets K_j slots (max in-degree over the 8 blocks dealt at step
    j; schedule shared by all cores so the SPMD program is identical).
  - A/B partition packing: slots are split into an A half and a B half.
    One PSUM tile [128, 512] holds h for 512 A-entries on partitions 0:64
    and 512 B-entries on partitions 64:128, via two accumulating matmuls
    with zero-padded stationaries [W_i | 0] and [0 | W_i].  All downstream
    vector/scalar ops then run at full 128-partition width.
  - relu + segment-sum are fused: scalar_tensor_tensor computes
    AG += max(psum, 0) per chunk (bf16 accumulator), then two halving adds
    collapse AG's 4 slot-columns; the A-half/B-half merge is folded into
    the tail matmul with a stacked stationary [W_ch ; W_ch].
  - the tail (W_ch, relu, W_o, * dis_d) runs once, batched over all 49
    blocks at 512-wide, entirely in bf16 (fp32 matmuls are 4x slower).

Per-core traffic is the ~28MB xseq stream; everything else is on-chip.
"""

import sys

sys.path.insert(0, "/opt/trn_rl_repo")

from contextlib import ExitStack

import ml_dtypes
import numpy as np

import concourse.bass as bass
import concourse.tile as tile
from concourse import bacc, mybir
from concourse.bass_utils import run_bass_kernel_spmd

F32 = mybir.dt.float32
BF16 = mybir.dt.bfloat16
AF = mybir.ActivationFunctionType
ALU = mybir.AluOpType

N_CORES = 8
P = 128
MMF = 512          # matmul moving free dim (4 slots of 128)


def _host_prep(x, edge_index, W_i, b_i, W_c, W_h, b_h, W_o, b_o):
    """Returns (in_maps, meta) for run_bass_kernel_spmd."""
    n, in_dim = x.shape
    hid = W_i.shape[1]
    npad = -(-n // 1024) * 1024
    nblkg = npad // P
    assert nblkg % N_CORES == 0
    nblk = nblkg // N_CORES

    row = np.concatenate([edge_index[0], np.arange(n)]).astype(np.int64)
    col = np.concatenate([edge_index[1], np.arange(n)]).astype(np.int64)

    outdeg = np.bincount(row, minlength=n).astype(np.float64)
    dis = (outdeg ** -0.5).astype(np.float32)   # deg >= 1 (self loops)

    indeg = np.bincount(col, minlength=npad)
    order = np.argsort(-indeg, kind="stable")   # dsts by in-degree desc
    dst_gp = order.reshape(nblkg, P)            # [global block, partition]
    kblk = indeg[order].reshape(nblkg, P).max(1)
    # blocks are in degree order; deal round-robin: step j gets blocks
    # j*8 .. j*8+7, K_j = max over them (tight since sorted)
    K = kblk.reshape(nblk, N_CORES).max(1).astype(np.int64)
    K = np.maximum(K, 1)
    # processing order: K descending (deep prefetch while compute ramps,
    # small blocks drain quickly at the end); a pyramid order measured worse
    perm = np.arange(nblk)
    K = K[perm]
    KH = -(-K // 2)                              # A/B pair-slots per block
    CW = 2 * KH * P                              # xseq columns per block
    cbase = np.concatenate([[0], np.cumsum(CW)])
    Ltot = int(cbase[-1])

    # edges sorted by destination; starts[d] = first edge of dst d
    e_order = np.argsort(col, kind="stable")
    csrc = row[e_order]
    starts = np.searchsorted(col[e_order], np.arange(npad))

    # per-(step, slot) tables, slot s of block j lives at xseq column
    #   cbase[j] + (sp // 4) * 1024 + half * wt + (sp % 4) * 128 + p
    # where sp = s if s < KH[j] (A half) else s - KH[j] (B half) and wt is
    # the chunk width (512, except the last partial chunk of a block)
    SKtot = int(K.sum())
    row_j = np.repeat(np.arange(nblk), K)            # [SKtot]
    row_s = np.arange(SKtot) - np.repeat(np.cumsum(K) - K, K)
    khj = KH[row_j]
    half = (row_s >= khj).astype(np.int64)
    sp = row_s - half * khj                          # pair-slot index
    wt = np.minimum(MMF, (khj - (sp // 4) * 4) * P)  # chunk width
    colpos = cbase[row_j] + (sp // 4) * 1024 + half * wt + (sp % 4) * P

    # dis-prescaled, transposed x with a zero pad column at index n
    xs_T = np.zeros((in_dim, n + 1), ml_dtypes.bfloat16)
    xs_T[:, :n] = (x.T * dis[None, :]).astype(ml_dtypes.bfloat16)

    dis_pad = np.zeros(npad, np.float32)
    dis_pad[:n] = dis

    has_bi = bool(np.any(np.asarray(b_i)))
    has_bh = bool(np.any(np.asarray(b_h)))

    in_maps = []
    gbs = []
    for c in range(N_CORES):
        gb = perm * N_CORES + c                      # global block ids
        gbs.append(gb)
        dsts = dst_gp[gb]                            # [nblk, P]
        dst_mat = dsts[row_j]                        # [SKtot, P]
        deg_mat = indeg[dst_mat]
        mask = row_s[:, None] < deg_mat              # valid slot?
        eidx = starts[dst_mat] + row_s[:, None]
        seq = np.where(mask, csrc[np.minimum(eidx, len(csrc) - 1)], n)
        seq_cols = np.full(Ltot, n, np.int64)        # default: zero pad col
        seq_cols[(colpos[:, None] + np.arange(P)).reshape(-1)] = seq.reshape(-1)
        xseq = np.ascontiguousarray(xs_T[:, seq_cols])
        dRow = dis_pad[dsts].reshape(1, nblk * P).astype(np.float32)
        m = {"xseq": xseq, "dRow": np.ascontiguousarray(dRow)}
        if has_bi:
            dseq_e = np.where(mask, dis[np.minimum(seq, n - 1)], 0.0)
            dseq = np.zeros(Ltot, np.float32)
            dseq[(colpos[:, None] + np.arange(P)).reshape(-1)] = dseq_e.reshape(-1)
            m["disSeq"] = dseq.reshape(1, Ltot)
        if has_bh:
            with np.errstate(divide="ignore"):
                invd = np.where(dRow > 0, 1.0 / np.maximum(dRow, 1e-30), 0.0)
            m["invdRow"] = invd.astype(np.float32)
        in_maps.append(m)

    W_ch = (np.asarray(W_c, np.float64) @ np.asarray(W_h, np.float64))
    Wi64 = np.asarray(W_i, np.float64)
    W_iA = np.concatenate([Wi64, np.zeros_like(Wi64)], axis=1)  # [W_i | 0]
    W_iB = np.concatenate([np.zeros_like(Wi64), Wi64], axis=1)  # [0 | W_i]
    W_chAB = np.concatenate([W_ch, W_ch], axis=0)               # [W_ch ; W_ch]
    shared = {
        "W_iA": np.ascontiguousarray(W_iA).astype(ml_dtypes.bfloat16),
        "W_iB": np.ascontiguousarray(W_iB).astype(ml_dtypes.bfloat16),
        "W_chAB": np.ascontiguousarray(W_chAB).astype(ml_dtypes.bfloat16),
        "W_o": np.asarray(W_o).astype(ml_dtypes.bfloat16),
    }
    if has_bi:
        shared["b_i"] = np.asarray(b_i, np.float32).reshape(1, hid)
    if has_bh:
        shared["b_h"] = np.asarray(b_h, np.float32).reshape(1, hid)
    for m in in_maps:
        m.update(shared)

    meta = {
        "n": n,
        "npad": npad,
        "nblk": nblk,
        "K": K,
        "KH": KH,
        "cbase": cbase,
        "Ltot": Ltot,
        "in_dim": in_dim,
        "hid": hid,
        "dst_gp": dst_gp,
        "gbs": gbs,
        "has_bi": has_bi,
        "has_bh": has_bh,
        "b_o": float(np.asarray(b_o).reshape(-1)[0]),
    }
    return in_maps, meta


def _build(meta):
    nblk = meta["nblk"]
    KH = meta["KH"]
    cbase = meta["cbase"]
    Ltot = meta["Ltot"]
    in_dim = meta["in_dim"]
    hid = meta["hid"]
    has_bi = meta["has_bi"]
    has_bh = meta["has_bh"]
    b_o = meta["b_o"]
    khmax = int(KH.max())
    NO = nblk * P                                  # output columns

    nc = bacc.Bacc()
    xseq = nc.declare_dram_parameter("xseq", [in_dim, Ltot], BF16, isOutput=False)
    W_iA = nc.declare_dram_parameter("W_iA", [in_dim, 2 * hid], BF16, isOutput=False)
    W_iB = nc.declare_dram_parameter("W_iB", [in_dim, 2 * hid], BF16, isOutput=False)
    W_chAB = nc.declare_dram_parameter("W_chAB", [2 * hid, hid], BF16,
                                       isOutput=False)
    W_o = nc.declare_dram_parameter("W_o", [hid, 1], BF16, isOutput=False)
    dRow = nc.declare_dram_parameter("dRow", [1, NO], F32, isOutput=False)
    if has_bi:
        b_i = nc.declare_dram_parameter("b_i", [1, hid], F32, isOutput=False)
        disSeq = nc.declare_dram_parameter("disSeq", [1, Ltot], F32, isOutput=False)
    if has_bh:
        b_h = nc.declare_dram_parameter("b_h", [1, hid], F32, isOutput=False)
        invdRow = nc.declare_dram_parameter("invdRow", [1, NO], F32, isOutput=False)
    out = nc.declare_dram_parameter("out", [1, NO], F32, isOutput=True)

    with tile.TileContext(nc) as tc, ExitStack() as ctx:
        singles = ctx.enter_context(tc.tile_pool(name="singles", bufs=1))
        sWiA = singles.tile([in_dim, 2 * hid], BF16)
        sWiB = singles.tile([in_dim, 2 * hid], BF16)
        sWch = singles.tile([2 * hid, hid], BF16)
        sWo = singles.tile([hid, 1], BF16)
        sdR = singles.tile([1, NO], F32)
        zall = singles.tile([2 * hid, NO], BF16)
        outrow = singles.tile([1, NO], F32)
        loads = [(sWiA, W_iA), (sWiB, W_iB), (sWch, W_chAB), (sWo, W_o),
                 (sdR, dRow)]
        if has_bi:
            sbi = singles.tile([1, hid], F32)
            sdis = singles.tile([1, Ltot], F32)
            loads += [(sbi, b_i), (sdis, disSeq)]
        if has_bh:
            sbh = singles.tile([1, hid], F32)
            sinvd = singles.tile([1, NO], F32)
            loads += [(sbh, b_h), (sinvd, invdRow)]
        for dst_t, src_t in loads:
            nc.sync.dma_start(out=dst_t[:], in_=src_t[:])

        with (
            tc.tile_pool(name="px", bufs=7) as px,
            tc.tile_pool(name="pps", bufs=4, space="PSUM") as pps,
            tc.tile_pool(name="pag", bufs=3) as pag,
            tc.tile_pool(name="ph", bufs=2) as ph,
            tc.tile_pool(name="ps2", bufs=2, space="PSUM") as ps2,
            tc.tile_pool(name="pso", bufs=2, space="PSUM") as pso,
        ):
            def tail_chunk(t):
                # tail over 4 blocks: W_ch (+A/B merge), relu, W_o, *dis
                w = min(MMF, NO - t)
                p2 = ps2.tile([hid, MMF], F32)
                nc.tensor.matmul(p2[:, :w], lhsT=sWch[:], rhs=zall[:, t: t + w],
                                 start=True, stop=not has_bh)
                if has_bh:
                    nc.tensor.matmul(p2[:, :w], lhsT=sbh[:],
                                     rhs=sinvd[:, t: t + w],
                                     start=False, stop=True)
                h2 = ph.tile([hid, MMF], BF16)
                nc.scalar.activation(h2[:, :w], p2[:, :w], AF.Relu, bias=0.0)
                po = pso.tile([1, MMF], F32)
                nc.tensor.matmul(po[:, :w], lhsT=sWo[:], rhs=h2[:, :w],
                                 start=True, stop=True)
                nc.vector.tensor_mul(outrow[:, t: t + w], po[:, :w],
                                     sdR[:, t: t + w])
                if b_o != 0.0:
                    nc.vector.tensor_scalar_add(
                        outrow[:, t: t + w], outrow[:, t: t + w], b_o,
                    )

            for j in range(nblk):
                KHj = int(KH[j])
                off = int(cbase[j])
                L = 2 * KHj * P                    # block columns (A+B)
                xb = px.tile([in_dim, 2 * khmax * P], BF16, tag="xb")
                dmae = (nc.sync, nc.scalar, nc.gpsimd)[j % 3]
                dmae.dma_start(out=xb[:, :L], in_=xseq[:, off: off + L])
                # Pool cannot read PSUM, so the STT chunks stay on DVE; the
                # SBUF-only collapse ops go to the otherwise-idle Pool engine
                ve = nc.gpsimd
                AG = pag.tile([P, MMF], BF16, tag="ag")
                nchunk = -(-KHj // 4)
                for t in range(nchunk):
                    w = min(MMF, KHj * P - t * MMF)
                    ca = t * 1024                  # A cols of this chunk
                    ps = pps.tile([P, MMF], F32)
                    nc.tensor.matmul(
                        ps[:, :w], lhsT=sWiA[:], rhs=xb[:, ca: ca + w],
                        start=True, stop=False,
                    )
                    nc.tensor.matmul(
                        ps[:, :w], lhsT=sWiB[:],
                        rhs=xb[:, ca + w: ca + 2 * w],
                        start=False, stop=not has_bi,
                    )
                    if has_bi:
                        # rank-1 bias: A then B half (disSeq is column-matched)
                        nc.tensor.matmul(
                            ps[:, :w], lhsT=sbi[:],
                            rhs=sdis[:, off + ca: off + ca + w],
                            start=False, stop=False,
                        )
                        nc.tensor.matmul(
                            ps[:, :w], lhsT=sbi[:],
                            rhs=sdis[:, off + ca + w: off + ca + 2 * w],
                            start=False, stop=True,
                        )
                    if t == 0:
                        nc.scalar.activation(AG[:, :w], ps[:, :w],
                                             AF.Relu, bias=0.0)
                    else:
                        nc.vector.scalar_tensor_tensor(
                            AG[:, :w], ps[:, :w], 0.0, AG[:, :w],
                            op0=ALU.max, op1=ALU.add,
                        )
                # collapse AG's remaining (up to 4) slot-columns; the last
                # halving step (on Pool, which is otherwise just issuing
                # DMAs) writes straight into zall
                zsl = zall[:, j * P: (j + 1) * P]
                k = min(KHj, 4)
                while k > 2:
                    k2 = k // 2
                    h = k - k2
                    nc.vector.tensor_add(
                        AG[:, : k2 * P], AG[:, : k2 * P],
                        AG[:, h * P: (h + k2) * P],
                    )
                    k = h
                if k == 2:
                    ve.tensor_add(zsl, AG[:, :P], AG[:, P: 2 * P])
                else:
                    ve.tensor_copy(zsl, AG[:, :P])
                if j % 4 == 3:
                    tail_chunk((j - 3) * P)
            if nblk % 4 != 0:
                tail_chunk((nblk - nblk % 4) * P)
        nc.sync.dma_start(out=out[:], in_=outrow[:])

    nc.finalize()
    return nc


def _assemble(results, meta):
    n = meta["n"]
    npad = meta["npad"]
    nblk = meta["nblk"]
    dst_gp = meta["dst_gp"]
    out_full = np.zeros(npad, np.float32)
    for c in range(N_CORES):
        vals = np.asarray(results[c]["out"]).reshape(nblk * P)
        out_full[dst_gp[meta["gbs"][c]].ravel()] = vals
    return out_full[:n].reshape(n, 1).astype(np.float32)


def kernel(x, edge_index, W_i, b_i, W_c, W_h, b_h, W_o, b_o):
    x = np.asarray(x)
    edge_index = np.asarray(edge_index)
    in_maps, meta = _host_prep(
        x, edge_index,
        np.asarray(W_i), np.asarray(b_i), np.asarray(W_c),
        np.asarray(W_h), np.asarray(b_h), np.asarray(W_o), np.asarray(b_o),
    )
    nc = _build(meta)
    res = run_bass_kernel_spmd(nc, in_maps, list(range(N_CORES)))
    return _assemble(res.results, meta)
